# revision 34
# baseline (speedup 1.0000x reference)
"""Trainium2 Bass kernel for nn_MemResProjections (memory-residual attention).

Reference computation (B=4, S=2048, K=64, H=1024, fp32):
    normed = rmsnorm(hidden) * norm_w
    v_pool = concat([normed, memory], axis=1)            # (B, S+K, H)
    q = normed @ q_w.T ; k = v_pool @ k_w.T
    logits = q @ k.T / sqrt(H)  with causal mask on the local S block,
    memory columns fully visible
    attn = softmax(logits); h~ = attn @ v_pool
    alpha = sigmoid(hidden @ gate_w.T + gate_b)
    out = (1-alpha)*hidden + alpha*h~

Algebraic folds:
  * logits = normed @ (q_w.T @ k_w) @ v_pool.T -> the k projection
    disappears. Host precomputes A3 = diag(w) (q_w.T @ k_w) diag(w) and
    q3 = xr @ A3 (xr = rstd-scaled hidden) contracts directly against the
    xr of the kv rows; memory rows use (memory / w) on both sides.
  * norm_w commutes through the attention sum: h~ = (attn @ [xr; mem/w]) * w,
    and the trailing *w is folded into alpha (alpha2 = alpha*w), so no
    per-kv-row scaling is needed at all. (Assumes norm_w has no zeros that
    coincide with nonzero memory columns -- setup_inputs uses ones.)

Sharding: 8 cores = (batch b, parity) pairs. Core owns the 8 even or odd
128-row query tiles of its batch -> causal work is balanced across the two
cores. The core's kv rows are laid out in "position" order (parity 1 swaps
tile pairs so its own queries sit at even positions); query tile p == kv
position 2p for both parities, so one uniform SPMD program with a single
per-core bias constant (pbias) handles both. Visibility of kv position
t for the query pair (2g, 2g+1):
    t < 4g: full   t==4g: [tri|0]   t==4g+1: [pb|0]
    t==4g+2: [-inf|tri]             t==4g+3: [-inf|pb]
with pb = -inf for parity 0, 0 for parity 1.

All matmul operands bf16 (psum f32); fully SBUF-resident. All DMAs ride the
sync queue (descriptor generation costs ~0.6us on the issuing engine, so
weight tensors load as single rearranged DMAs and small consts are packed).
Sqrt/Sigmoid run batched (4 stripes) to limit activation-table swaps; h~
psum is evicted with an unscaled copy so the next group's accumulation can
start before the softmax denominator arrives.
"""
import numpy as np
import ml_dtypes

import concourse.bass as bass
import concourse.mybir as mybir
import concourse.tile as tile
from concourse.bass_utils import run_bass_kernel_spmd

F32 = mybir.dt.float32
F16 = mybir.dt.float16
BF16 = mybir.dt.bfloat16
AFT = mybir.ActivationFunctionType
ALU = mybir.AluOpType
NPBF = ml_dtypes.bfloat16

P = 128
H = 1024
NJ = H // P           # hidden-dim 128-blocks
T_MEM = 64
SCALE = 1.0 / 32.0    # 1/sqrt(H)
EPS = 1e-6
NEG = -1.0e30

N_CORES = 8
B_FULL, S_FULL = 4, 2048


# ---------------------------------------------------------------- walrus fix
ENGINE_ATTR = {
    mybir.EngineType.PE: "tensor",
    mybir.EngineType.Activation: "scalar",
    mybir.EngineType.DVE: "vector",
    mybir.EngineType.Pool: "gpsimd",
    mybir.EngineType.SP: "sync",
}
DMA_OPS = ("InstDMACopy", "InstDMATranspose", "InstTensorLoad", "InstTensorSave",
           "InstCollectiveCompute")


def split_multi_waits(nc, limit=1, dma_limit=None):
    """This walrus build rejects engine instructions carrying more than one
    sem wait (and any wait on a DMA transpose); hoist extras onto same-engine
    NOPs inserted just before."""
    n_split = 0
    for f in nc.m.functions:
        for blk in f.blocks:
            il = blk.instructions
            i = 0
            while i < len(il):
                ins = il[i]
                tname = type(ins).__name__
                if tname == "InstDmaTransposeAnt":
                    lim = 0
                else:
                    lim = dma_limit if tname in DMA_OPS else limit
                si = ins.sync_info
                waits = list(si.on_wait) if si is not None and si.on_wait else []
                if lim is not None and len(waits) > lim:
                    keep, extra = waits[:lim], waits[lim:]
                    si.on_wait.clear()
                    for w in keep:
                        si.on_wait.append(w)
                    eng = getattr(nc, ENGINE_ATTR[ins.engine])
                    for w in extra:
                        nop = eng.nop(nofuse=True, hint="wait_split")
                        nop.wait_op(bass.SemaphoreHandle(w.ant_name, w.id),
                                    w.wait_value, "sem-ge")
                        popped = nc.cur_bb.bb.instructions.pop()
                        assert popped.name == nop.ins.name
                        il.insert(i, nop.ins)
                        i += 1
                        n_split += 1
                i += 1
    return n_split


def pos_col(t):
    """normedT column block of kv position t (query stripes packed first)."""
    return (t // 2) * P if t % 2 == 0 else 1024 + (t // 2) * P


# ---------------------------------------------------------------- program
def build_nc():
    nc = bass.Bass()
    x_all = nc.declare_dram_parameter("x_all", [S_FULL, H], F16, isOutput=False)
    # packed consts: cf32 = [pbias | tri], cbf = [ones | b_bc]
    cf32_d = nc.declare_dram_parameter("cf32", [P, 1 + P], F32, isOutput=False)
    cbf_d = nc.declare_dram_parameter("cbf", [P, 1 + H], BF16, isOutput=False)
    wbc_d = nc.declare_dram_parameter("w_bc", [P, H], BF16, isOutput=False)
    vmem_d = nc.declare_dram_parameter("vmem", [T_MEM, H], BF16, isOutput=False)
    memT3_d = nc.declare_dram_parameter("memT3", [H, T_MEM], BF16, isOutput=False)
    a3_d = nc.declare_dram_parameter("a3", [H, H], BF16, isOutput=False)
    gw_d = nc.declare_dram_parameter("gw", [H, H], BF16, isOutput=False)
    out = nc.declare_dram_parameter("out", [S_FULL // 2, H], F32, isOutput=True)

    with tile.TileContext(nc) as tc:
        from contextlib import ExitStack
        with ExitStack() as ctx:
            const = ctx.enter_context(tc.tile_pool(name="const", bufs=1))

            eps_t = const.tile([P, 1], F32)
            nc.vector.memset(eps_t[:], EPS)
            onesf = const.tile([P, 1], F32)
            nc.vector.memset(onesf[:], 1.0)
            cf32 = const.tile([P, 1 + P], F32)
            pb_t = cf32[:, 0:1]
            tri_t = cf32[:, 1:1 + P]
            cbf = const.tile([P, 1 + H], BF16)
            ones_col = cbf[:, 0:1]
            b_bc_t = cbf[:, 1:1 + H]
            w_bc_t = const.tile([P, H], BF16)   # norm_w broadcast (alpha fold)

            ss_all = const.tile([P, 16], F32)
            std_all = const.tile([P, 16], F32)
            rstd_all = const.tile([P, 16], F32)
            rden = const.tile([P, 8], F32)

            normedT = const.tile([P, NJ, 2048], BF16)   # xr^T, H on partitions
            qT = const.tile([P, NJ, H], BF16)           # q3^T for own queries
            vnat = [const.tile([P, H], BF16, tag=f"vnat{t}", name=f"vnat{t}")
                    for t in range(16)]                 # xr rows by kv position
            alpha = [const.tile([P, H], F16, tag=f"alpha{p}", name=f"alpha{p}")
                     for p in range(8)]                 # logits -> alpha -> alpha*w
            xq = [const.tile([P, H], F16, tag=f"xq{p}", name=f"xq{p}")
                  for p in range(8)]                    # x, later (alpha-1)*x

            a3_t = const.tile([P, NJ, H], BF16)
            gw_t = const.tile([P, NJ, H], BF16)
            memT3_t = const.tile([P, NJ, T_MEM], BF16)
            vmem_t = const.tile([T_MEM, H], BF16)

            # ============ phase A: stats, transpose, gate, q3 projection
            with tc.tile_pool(name="ast", bufs=2) as ast, \
                 tc.tile_pool(name="apsG", bufs=2, space="PSUM") as apsG:
                sq = ast.tile([P, H], BF16, bufs=1)     # Square scratch (discarded)
                # sync ring in need-order: all x stripes burst first, then
                # gate weights, small consts, then the later-phase tensors
                xt_of = {}
                for idx in range(16):
                    t = 2 * idx if idx < 8 else 2 * (idx - 8) + 1
                    xt = xq[idx] if idx < 8 else ast.tile([P, H], F16,
                                                          tag="xt", bufs=8)
                    xt_of[idx] = xt
                    nc.sync.dma_start(out=xt[:], in_=x_all[t * P:(t + 1) * P, :])
                nc.sync.dma_start(out=gw_t[:, :, :],
                                  in_=gw_d[:].rearrange("(j p) h -> p j h", p=P))
                nc.sync.dma_start(out=cbf[:], in_=cbf_d[:])
                nc.sync.dma_start(out=w_bc_t[:], in_=wbc_d[:])
                nc.sync.dma_start(out=cf32[:], in_=cf32_d[:])
                nc.sync.dma_start(out=a3_t[:, :, :],
                                  in_=a3_d[:].rearrange("(j p) h -> p j h", p=P))
                nc.sync.dma_start(
                    out=memT3_t[:, :, :],
                    in_=memT3_d[:].rearrange("(j p) t -> p j t", p=P))
                nc.sync.dma_start(out=vmem_t[:], in_=vmem_d[:])

                batches = [[0, 1], [2, 3], [4, 5, 6, 7], [8, 9, 10, 11],
                           [12, 13, 14, 15]]
                for batch, idxs in enumerate(batches):
                    for idx in idxs:
                        xt = xt_of[idx]
                        nc.scalar.activation(sq[:], xt[:], AFT.Square,
                                             accum_out=ss_all[:, idx:idx + 1])
                    # batched sqrt (one table swap), reciprocal
                    c4 = slice(idxs[0], idxs[-1] + 1)
                    nc.scalar.activation(std_all[:, c4], ss_all[:, c4], AFT.Sqrt,
                                         scale=1.0 / H, bias=eps_t[:])
                    nc.vector.reciprocal(rstd_all[:, c4], std_all[:, c4])
                    # normalize, XBAR-transpose into normedT, gate logits
                    for idx in idxs:
                        t = 2 * idx if idx < 8 else 2 * (idx - 8) + 1
                        is_q = idx < 8
                        xt = xt_of[idx]
                        rsc = rstd_all[:, idx:idx + 1]
                        nc.vector.tensor_scalar_mul(vnat[t][:], xt[:], rsc)
                        c = idx * P
                        nc.scalar.dma_start_transpose(normedT[:, :, c:c + P],
                                                      vnat[t][:])
                        if is_q:
                            for oc in range(2):
                                pg = apsG.tile([P, 512], F32, tag="pg")
                                for j in range(NJ):
                                    nc.tensor.matmul(
                                        pg[:], normedT[:, j, c:c + P],
                                        gw_t[:, j, oc * 512:(oc + 1) * 512],
                                        start=(j == 0), stop=(j == NJ - 1))
                                # gate logits (f16) into the alpha tile
                                nc.vector.scalar_tensor_tensor(
                                    alpha[idx][:, oc * 512:(oc + 1) * 512],
                                    pg[:], std_all[:, idx:idx + 1],
                                    b_bc_t[:, oc * 512:(oc + 1) * 512],
                                    op0=ALU.mult, op1=ALU.add)
                    # batched sigmoids, then (alpha-1)*x
                    if idxs[0] < 8:
                        for idx in idxs:
                            nc.scalar.activation(alpha[idx][:], alpha[idx][:],
                                                 AFT.Sigmoid)
                        for idx in idxs:
                            nc.vector.scalar_tensor_tensor(
                                xq[idx][:], alpha[idx][:], 1.0, xq[idx][:],
                                op0=ALU.subtract, op1=ALU.mult)

                # alpha <- alpha*w on the idle Pool engine (needed only by
                # the B-phase combine)
                for idx in range(8):
                    nc.gpsimd.tensor_mul(alpha[idx][:], alpha[idx][:],
                                         w_bc_t[:])

                # q3^T projection (even normedT columns = own queries)
                with tc.tile_pool(name="apsQ", bufs=2, space="PSUM") as apsQ:
                    for half in range(2):
                        for m in range(NJ):
                            pq = apsQ.tile([P, 512], F32, tag="pq")
                            for j in range(NJ):
                                nc.tensor.matmul(
                                    pq[:], a3_t[:, j, m * P:(m + 1) * P],
                                    normedT[:, j, half * 512:(half + 1) * 512],
                                    start=(j == 0), stop=(j == NJ - 1))
                            if m % 2 == 0:
                                nc.scalar.activation(
                                    qT[:, m, half * 512:(half + 1) * 512],
                                    pq[:], AFT.Copy)
                            else:
                                nc.vector.tensor_copy(
                                    qT[:, m, half * 512:(half + 1) * 512], pq[:])

            # ============ phase B: attention + combine
            with tc.tile_pool(name="bst", bufs=2) as bst, \
                 tc.tile_pool(name="bet", bufs=3) as bet, \
                 tc.tile_pool(name="bps", bufs=2, space="PSUM") as bps, \
                 tc.tile_pool(name="bph", bufs=1, space="PSUM") as bph, \
                 tc.tile_pool(name="bpd", bufs=1, space="PSUM") as bpd, \
                 tc.tile_pool(name="bpd2", bufs=1, space="PSUM") as bpd2:
                for g in range(4):
                    scol = g * 256
                    ntau = 4 * g + 4
                    ph = {(sl, hc): bph.tile([P, 512], F32, tag=f"ph{sl}{hc}",
                                             name=f"ph{sl}{hc}")
                          for sl in range(2) for hc in range(2)}
                    pd_row = bpd.tile([1, 256], F32, tag="pdrow")
                    for ti in range(ntau + 1):
                        is_mem = ti == ntau
                        rows = T_MEM if is_mem else P
                        first, last = ti == 0, is_mem
                        # left query half fully masked for the last two own taus
                        half_only = (not is_mem) and ti >= 4 * g + 2
                        c0 = P if half_only else 0
                        ps = bps.tile([P, 256], F32, tag="ps")
                        for j in range(NJ):
                            lhs = (memT3_t[:, j, :] if is_mem
                                   else normedT[:, j, pos_col(ti):pos_col(ti) + P])
                            nc.tensor.matmul(ps[:rows, c0:256], lhs,
                                             qT[:, j, scol + c0:scol + 256],
                                             start=(j == 0), stop=(j == NJ - 1))
                        et = bet.tile([P, 256], BF16, tag="et")
                        if is_mem or ti < 4 * g:
                            nc.scalar.activation(et[:rows, :], ps[:rows, :],
                                                 AFT.Exp, scale=SCALE)
                        elif ti == 4 * g:
                            nc.vector.tensor_add(ps[:, 0:P], ps[:, 0:P], tri_t)
                            nc.scalar.activation(et[:], ps[:], AFT.Exp,
                                                 scale=SCALE)
                        elif ti == 4 * g + 1:
                            nc.scalar.activation(et[:, 0:P], ps[:, 0:P], AFT.Exp,
                                                 scale=SCALE, bias=pb_t)
                            nc.scalar.activation(et[:, P:256], ps[:, P:256],
                                                 AFT.Exp, scale=SCALE)
                        elif ti == 4 * g + 2:
                            nc.vector.tensor_add(ps[:, P:256], ps[:, P:256],
                                                 tri_t)
                            nc.scalar.activation(et[:, P:256], ps[:, P:256],
                                                 AFT.Exp, scale=SCALE)
                        else:  # ti == 4*g + 3
                            nc.scalar.activation(et[:, P:256], ps[:, P:256],
                                                 AFT.Exp, scale=SCALE, bias=pb_t)
                        vt = vmem_t if is_mem else vnat[ti]
                        for sl in range(2):
                            if half_only and sl == 0:
                                continue
                            for hc in range(2):
                                nc.tensor.matmul(
                                    ph[(sl, hc)][:],
                                    et[:rows, sl * P:(sl + 1) * P],
                                    vt[:rows, hc * 512:(hc + 1) * 512],
                                    start=first, stop=last,
                                    skip_group_check=True)
                        nc.tensor.matmul(pd_row[0:1, c0:256],
                                         ones_col[:rows, :],
                                         et[:rows, c0:256], start=first,
                                         stop=last, skip_group_check=True)
                    # raw-copy h~ out of psum (releases banks immediately)
                    hs = {}
                    for sl in range(2):
                        for hc in range(2):
                            hsb = bst.tile([P, 512], F32, tag="hsb", bufs=8)
                            nc.vector.tensor_copy(hsb[:], ph[(sl, hc)][:])
                            hs[(sl, hc)] = hsb
                    # denominator [1,256] -> [128,2] via PE transpose, recip
                    pdsb = bst.tile([1, 256], F32, tag="pdsb")
                    nc.vector.tensor_copy(pdsb[:], pd_row[:])
                    pdt = bpd2.tile([P, 2], F32, tag="pdt")
                    for sl in range(2):
                        nc.tensor.transpose(pdt[:, sl:sl + 1],
                                            pdsb[0:1, sl * P:(sl + 1) * P],
                                            onesf[0:1, 0:1])
                    nc.vector.reciprocal(rden[:, g * 2:g * 2 + 2], pdt[:])
                    # combine: out = (h~raw*rden)*(alpha*w) - (alpha-1)*x
                    for sl in range(2):
                        p = 2 * g + sl
                        outb = bst.tile([P, H], F32, tag="outb")
                        for hc in range(2):
                            tmp = bst.tile([P, 512], F32, tag="tmp")
                            nc.vector.scalar_tensor_tensor(
                                tmp[:], hs[(sl, hc)][:],
                                rden[:, g * 2 + sl:g * 2 + sl + 1],
                                alpha[p][:, hc * 512:(hc + 1) * 512],
                                op0=ALU.mult, op1=ALU.mult)
                            sub_eng = nc.vector if g == 3 else nc.gpsimd
                            sub_eng.tensor_sub(
                                outb[:, hc * 512:(hc + 1) * 512], tmp[:],
                                xq[p][:, hc * 512:(hc + 1) * 512])
                        nc.sync.dma_start(out=out[p * P:(p + 1) * P, :],
                                          in_=outb[:])

    import os
    if os.environ.get("NO_WAIT_SPLIT") != "1":
        split_multi_waits(nc, limit=1, dma_limit=1)
    return nc


_NC_CACHE = None
_LAST_IN_MAPS = None


def _get_nc():
    global _NC_CACHE
    if _NC_CACHE is None:
        _NC_CACHE = build_nc()
    return _NC_CACHE


def prepare_in_maps(hidden_states, memory_state, q_w, k_w, norm_w, gate_w,
                    gate_b):
    hidden_states = np.asarray(hidden_states, dtype=np.float32)
    memory_state = np.asarray(memory_state, dtype=np.float32)
    q_w = np.asarray(q_w, dtype=np.float32)
    k_w = np.asarray(k_w, dtype=np.float32)
    norm_w = np.asarray(norm_w, dtype=np.float32)
    gate_w = np.asarray(gate_w, dtype=np.float32)
    gate_b = np.asarray(gate_b, dtype=np.float32)

    wsafe = np.where(norm_w == 0, np.float32(1.0), norm_w)
    a3 = np.ascontiguousarray(
        ((norm_w[:, None] * (q_w.T @ k_w)) * norm_w[None, :]).astype(NPBF))
    gw = np.ascontiguousarray(gate_w.T.astype(NPBF))
    tri = np.where(np.arange(P)[None, :] >= np.arange(P)[:, None],
                   np.float32(0.0), np.float32(NEG)).astype(np.float32)
    cbf = np.concatenate([np.ones((P, 1), NPBF),
                          np.broadcast_to(gate_b, (P, H)).astype(NPBF)], axis=1)
    cbf = np.ascontiguousarray(cbf)
    w_bc = np.ascontiguousarray(np.broadcast_to(norm_w, (P, H)).astype(NPBF))

    in_maps = []
    for c in range(N_CORES):
        b, par = divmod(c, 2)
        x16 = hidden_states[b].reshape(16, P, H)
        if par == 1:
            x16 = x16[[i ^ 1 for i in range(16)]]
        memb = memory_state[b] / wsafe[None, :]   # /w on both sides; *w folded
        cf32 = np.concatenate(
            [np.full((P, 1), NEG if par == 0 else 0.0, np.float32), tri], axis=1)
        in_maps.append({
            "x_all": np.ascontiguousarray(x16.reshape(S_FULL, H)
                                          .astype(np.float16)),
            "vmem": np.ascontiguousarray(memb.astype(NPBF)),
            "memT3": np.ascontiguousarray(memb.T.astype(NPBF)),
            "a3": a3, "gw": gw, "w_bc": w_bc,
            "cf32": np.ascontiguousarray(cf32), "cbf": cbf,
        })
    return in_maps


def kernel(**inputs):
    in_maps = prepare_in_maps(**inputs)
    global _LAST_IN_MAPS
    _LAST_IN_MAPS = in_maps
    nc = _get_nc()
    res = run_bass_kernel_spmd(nc, in_maps, list(range(N_CORES)))
    out = np.empty((B_FULL, S_FULL, H), dtype=np.float32)
    for c in range(N_CORES):
        b, par = divmod(c, 2)
        out[b].reshape(16, P, H)[par::2] = res.results[c]["out"].reshape(8, P, H)
    return out


# revision 37
# speedup vs baseline: 1.1983x; 1.1983x over previous
"""Trainium2 Bass kernel for nn_MemResProjections (memory-residual attention).

Reference computation (B=4, S=2048, K=64, H=1024, fp32):
    normed = rmsnorm(hidden) * norm_w
    v_pool = concat([normed, memory], axis=1)            # (B, S+K, H)
    q = normed @ q_w.T ; k = v_pool @ k_w.T
    logits = q @ k.T / sqrt(H)  with causal mask on the local S block,
    memory columns fully visible
    attn = softmax(logits); h~ = attn @ v_pool
    alpha = sigmoid(hidden @ gate_w.T + gate_b)
    out = (1-alpha)*hidden + alpha*h~

Algebraic folds:
  * logits = normed @ (q_w.T @ k_w) @ v_pool.T -> the k projection
    disappears. Host precomputes A3 = diag(w) (q_w.T @ k_w) diag(w) and
    q3 = xr @ A3 (xr = rstd-scaled hidden) contracts directly against the
    xr of the kv rows; memory rows use (memory / w) on both sides.
  * norm_w commutes through the attention sum: h~ = (attn @ [xr; mem/w]) * w,
    and the trailing *w is folded into alpha (alpha2 = alpha*w), so no
    per-kv-row scaling is needed at all. (Assumes norm_w has no zeros that
    coincide with nonzero memory columns -- setup_inputs uses ones.)

Sharding: 8 cores = (batch b, parity) pairs. Core owns the 8 even or odd
128-row query tiles of its batch -> causal work is balanced across the two
cores. The core's kv rows are laid out in "position" order (parity 1 swaps
tile pairs so its own queries sit at even positions); query tile p == kv
position 2p for both parities, so one uniform SPMD program with a single
per-core bias constant (pbias) handles both. Visibility of kv position
t for the query pair (2g, 2g+1):
    t < 4g: full   t==4g: [tri|0]   t==4g+1: [pb|0]
    t==4g+2: [-inf|tri]             t==4g+3: [-inf|pb]
with pb = -inf for parity 0, 0 for parity 1.

All matmul operands bf16 (psum f32); fully SBUF-resident. All DMAs ride the
sync queue (descriptor generation costs ~0.6us on the issuing engine, so
weight tensors load as single rearranged DMAs and small consts are packed).
Sqrt/Sigmoid run batched (4 stripes) to limit activation-table swaps; h~
psum is evicted with an unscaled copy so the next group's accumulation can
start before the softmax denominator arrives.
"""
import numpy as np
import ml_dtypes

import concourse.bass as bass
import concourse.mybir as mybir
import concourse.tile as tile
from concourse.bass_utils import run_bass_kernel_spmd

F32 = mybir.dt.float32
F16 = mybir.dt.float16
BF16 = mybir.dt.bfloat16
AFT = mybir.ActivationFunctionType
ALU = mybir.AluOpType
NPBF = ml_dtypes.bfloat16

P = 128
H = 1024
NJ = H // P           # hidden-dim 128-blocks
T_MEM = 64
SCALE = 1.0 / 32.0    # 1/sqrt(H)
EPS = 1e-6
NEG = -1.0e30

N_CORES = 8
B_FULL, S_FULL = 4, 2048


# ---------------------------------------------------------------- walrus fix
ENGINE_ATTR = {
    mybir.EngineType.PE: "tensor",
    mybir.EngineType.Activation: "scalar",
    mybir.EngineType.DVE: "vector",
    mybir.EngineType.Pool: "gpsimd",
    mybir.EngineType.SP: "sync",
}
DMA_OPS = ("InstDMACopy", "InstDMATranspose", "InstTensorLoad", "InstTensorSave",
           "InstCollectiveCompute")


def split_multi_waits(nc, limit=1, dma_limit=None):
    """This walrus build rejects engine instructions carrying more than one
    sem wait (and any wait on a DMA transpose); hoist extras onto same-engine
    NOPs inserted just before."""
    n_split = 0
    for f in nc.m.functions:
        for blk in f.blocks:
            il = blk.instructions
            i = 0
            while i < len(il):
                ins = il[i]
                tname = type(ins).__name__
                if tname == "InstDmaTransposeAnt":
                    lim = 0
                else:
                    lim = dma_limit if tname in DMA_OPS else limit
                si = ins.sync_info
                waits = list(si.on_wait) if si is not None and si.on_wait else []
                if lim is not None and len(waits) > lim:
                    keep, extra = waits[:lim], waits[lim:]
                    si.on_wait.clear()
                    for w in keep:
                        si.on_wait.append(w)
                    eng = getattr(nc, ENGINE_ATTR[ins.engine])
                    for w in extra:
                        nop = eng.nop(nofuse=True, hint="wait_split")
                        nop.wait_op(bass.SemaphoreHandle(w.ant_name, w.id),
                                    w.wait_value, "sem-ge")
                        popped = nc.cur_bb.bb.instructions.pop()
                        assert popped.name == nop.ins.name
                        il.insert(i, nop.ins)
                        i += 1
                        n_split += 1
                i += 1
    return n_split


def pos_col(t):
    """normedT column block of kv position t (query stripes packed first)."""
    return (t // 2) * P if t % 2 == 0 else 1024 + (t // 2) * P


# ---------------------------------------------------------------- program
def build_nc():
    nc = bass.Bass()
    x_all = nc.declare_dram_parameter("x_all", [S_FULL, H], F16, isOutput=False)
    # packed consts: cf32 = [pbias | tri], cbf = [ones | b_bc]
    cf32_d = nc.declare_dram_parameter("cf32", [P, 1 + P], F32, isOutput=False)
    cbf_d = nc.declare_dram_parameter("cbf", [P, 1 + H], BF16, isOutput=False)
    wbc_d = nc.declare_dram_parameter("w_bc", [P, H], BF16, isOutput=False)
    vmem_d = nc.declare_dram_parameter("vmem", [T_MEM, H], BF16, isOutput=False)
    memT3_d = nc.declare_dram_parameter("memT3", [H, T_MEM], BF16, isOutput=False)
    a3_d = nc.declare_dram_parameter("a3", [H, H], BF16, isOutput=False)
    gw_d = nc.declare_dram_parameter("gw", [H, H], BF16, isOutput=False)
    out = nc.declare_dram_parameter("out", [S_FULL // 2, H], F32, isOutput=True)

    with tile.TileContext(nc) as tc:
        from contextlib import ExitStack
        with ExitStack() as ctx:
            const = ctx.enter_context(tc.tile_pool(name="const", bufs=1))

            eps_t = const.tile([P, 1], F32)
            nc.vector.memset(eps_t[:], EPS)
            onesf = const.tile([P, 1], F32)
            nc.vector.memset(onesf[:], 1.0)
            cf32 = const.tile([P, 1 + P], F32)
            pb_t = cf32[:, 0:1]
            tri_t = cf32[:, 1:1 + P]
            cbf = const.tile([P, 1 + H], BF16)
            ones_col = cbf[:, 0:1]
            b_bc_t = cbf[:, 1:1 + H]
            w_bc_t = const.tile([P, H], BF16)   # norm_w broadcast (alpha fold)

            ss_all = const.tile([P, 16], F32)
            std_all = const.tile([P, 16], F32)
            rstd_all = const.tile([P, 16], F32)
            rden = const.tile([P, 8], F32)

            normedT = const.tile([P, NJ, 2048], BF16)   # xr^T, H on partitions
            qT = const.tile([P, NJ, H], BF16)           # q3^T for own queries
            vnat = [const.tile([P, H], BF16, tag=f"vnat{t}", name=f"vnat{t}")
                    for t in range(16)]                 # xr rows by kv position
            alpha = [const.tile([P, H], F16, tag=f"alpha{p}", name=f"alpha{p}")
                     for p in range(8)]                 # logits -> alpha -> alpha*w
            xq = [const.tile([P, H], F16, tag=f"xq{p}", name=f"xq{p}")
                  for p in range(8)]                    # x, later (alpha-1)*x

            a3_t = const.tile([P, NJ, H], BF16)
            gw_t = const.tile([P, NJ, H], BF16)
            memT3_t = const.tile([P, NJ, T_MEM], BF16)
            vmem_t = const.tile([T_MEM, H], BF16)

            # ============ phase A: stats, transpose, gate, q3 projection
            with tc.tile_pool(name="ast", bufs=2) as ast, \
                 tc.tile_pool(name="apsG", bufs=2, space="PSUM") as apsG:
                sq = ast.tile([P, H], BF16, bufs=1)     # Square scratch (discarded)
                # sync ring: gate weights lead (their transfer hides under the
                # stats ramp), then all x stripes burst, consts, and the
                # remaining tensors slot in between the (stalling) transposes
                nc.sync.dma_start(out=gw_t[:, :, :],
                                  in_=gw_d[:].rearrange("(j p) h -> p j h", p=P))
                xt_of = {}
                for idx in range(16):
                    t = 2 * idx if idx < 8 else 2 * (idx - 8) + 1
                    xt = xq[idx] if idx < 8 else ast.tile([P, H], F16,
                                                          tag="xt", bufs=8)
                    xt_of[idx] = xt
                    nc.sync.dma_start(out=xt[:], in_=x_all[t * P:(t + 1) * P, :])
                nc.sync.dma_start(out=cbf[:], in_=cbf_d[:])
                nc.sync.dma_start(out=w_bc_t[:], in_=wbc_d[:])
                nc.sync.dma_start(out=cf32[:], in_=cf32_d[:])

                batches = [[0, 1], [2, 3], [4, 5, 6, 7], [8, 9, 10, 11],
                           [12, 13, 14, 15]]
                for batch, idxs in enumerate(batches):
                    if batch == 2:
                        nc.sync.dma_start(
                            out=a3_t[:, :, :],
                            in_=a3_d[:].rearrange("(j p) h -> p j h", p=P))
                    elif batch == 3:
                        nc.sync.dma_start(
                            out=memT3_t[:, :, :],
                            in_=memT3_d[:].rearrange("(j p) t -> p j t", p=P))
                        nc.sync.dma_start(out=vmem_t[:], in_=vmem_d[:])
                    for idx in idxs:
                        xt = xt_of[idx]
                        nc.scalar.activation(sq[:], xt[:], AFT.Square,
                                             accum_out=ss_all[:, idx:idx + 1])
                    # batched sqrt (one table swap), reciprocal
                    c4 = slice(idxs[0], idxs[-1] + 1)
                    nc.scalar.activation(std_all[:, c4], ss_all[:, c4], AFT.Sqrt,
                                         scale=1.0 / H, bias=eps_t[:])
                    nc.vector.reciprocal(rstd_all[:, c4], std_all[:, c4])
                    # normalize, XBAR-transpose into normedT, gate logits
                    for idx in idxs:
                        t = 2 * idx if idx < 8 else 2 * (idx - 8) + 1
                        is_q = idx < 8
                        xt = xt_of[idx]
                        rsc = rstd_all[:, idx:idx + 1]
                        nc.vector.tensor_scalar_mul(vnat[t][:], xt[:], rsc)
                        c = idx * P
                        nc.sync.dma_start_transpose(normedT[:, :, c:c + P],
                                                    vnat[t][:])
                        if is_q:
                            for oc in range(2):
                                pg = apsG.tile([P, 512], F32, tag="pg")
                                for j in range(NJ):
                                    nc.tensor.matmul(
                                        pg[:], normedT[:, j, c:c + P],
                                        gw_t[:, j, oc * 512:(oc + 1) * 512],
                                        start=(j == 0), stop=(j == NJ - 1))
                                # gate logits (f16) into the alpha tile
                                nc.vector.scalar_tensor_tensor(
                                    alpha[idx][:, oc * 512:(oc + 1) * 512],
                                    pg[:], std_all[:, idx:idx + 1],
                                    b_bc_t[:, oc * 512:(oc + 1) * 512],
                                    op0=ALU.mult, op1=ALU.add)
                    # batched sigmoids, then (alpha-1)*x
                    if idxs[0] < 8:
                        for idx in idxs:
                            nc.scalar.activation(alpha[idx][:], alpha[idx][:],
                                                 AFT.Sigmoid)
                        for idx in idxs:
                            nc.vector.scalar_tensor_tensor(
                                xq[idx][:], alpha[idx][:], 1.0, xq[idx][:],
                                op0=ALU.subtract, op1=ALU.mult)

                # alpha <- alpha*w on the idle Pool engine (needed only by
                # the B-phase combine)
                for idx in range(8):
                    nc.gpsimd.tensor_mul(alpha[idx][:], alpha[idx][:],
                                         w_bc_t[:])

                # q3^T projection (even normedT columns = own queries)
                with tc.tile_pool(name="apsQ", bufs=2, space="PSUM") as apsQ:
                    for half in range(2):
                        for m in range(NJ):
                            pq = apsQ.tile([P, 512], F32, tag="pq")
                            for j in range(NJ):
                                nc.tensor.matmul(
                                    pq[:], a3_t[:, j, m * P:(m + 1) * P],
                                    normedT[:, j, half * 512:(half + 1) * 512],
                                    start=(j == 0), stop=(j == NJ - 1))
                            if m % 2 == 0:
                                nc.scalar.activation(
                                    qT[:, m, half * 512:(half + 1) * 512],
                                    pq[:], AFT.Copy)
                            else:
                                nc.vector.tensor_copy(
                                    qT[:, m, half * 512:(half + 1) * 512], pq[:])

            # ============ phase B: attention + combine
            with tc.tile_pool(name="bst", bufs=2) as bst, \
                 tc.tile_pool(name="bet", bufs=3) as bet, \
                 tc.tile_pool(name="bps", bufs=2, space="PSUM") as bps, \
                 tc.tile_pool(name="bph", bufs=1, space="PSUM") as bph, \
                 tc.tile_pool(name="bpd", bufs=1, space="PSUM") as bpd, \
                 tc.tile_pool(name="bpd2", bufs=1, space="PSUM") as bpd2:
                for g in range(4):
                    scol = g * 256
                    ntau = 4 * g + 4
                    ph = {(sl, hc): bph.tile([P, 512], F32, tag=f"ph{sl}{hc}",
                                             name=f"ph{sl}{hc}")
                          for sl in range(2) for hc in range(2)}
                    pd_row = bpd.tile([1, 256], F32, tag="pdrow")
                    for ti in range(ntau + 1):
                        is_mem = ti == ntau
                        rows = T_MEM if is_mem else P
                        first, last = ti == 0, is_mem
                        # left query half fully masked for the last two own taus
                        half_only = (not is_mem) and ti >= 4 * g + 2
                        c0 = P if half_only else 0
                        ps = bps.tile([P, 256], F32, tag="ps")
                        for j in range(NJ):
                            lhs = (memT3_t[:, j, :] if is_mem
                                   else normedT[:, j, pos_col(ti):pos_col(ti) + P])
                            nc.tensor.matmul(ps[:rows, c0:256], lhs,
                                             qT[:, j, scol + c0:scol + 256],
                                             start=(j == 0), stop=(j == NJ - 1))
                        et = bet.tile([P, 256], BF16, tag="et")
                        if is_mem or ti < 4 * g:
                            nc.scalar.activation(et[:rows, :], ps[:rows, :],
                                                 AFT.Exp, scale=SCALE)
                        elif ti == 4 * g:
                            nc.vector.tensor_add(ps[:, 0:P], ps[:, 0:P], tri_t)
                            nc.scalar.activation(et[:], ps[:], AFT.Exp,
                                                 scale=SCALE)
                        elif ti == 4 * g + 1:
                            nc.scalar.activation(et[:, 0:P], ps[:, 0:P], AFT.Exp,
                                                 scale=SCALE, bias=pb_t)
                            nc.scalar.activation(et[:, P:256], ps[:, P:256],
                                                 AFT.Exp, scale=SCALE)
                        elif ti == 4 * g + 2:
                            nc.vector.tensor_add(ps[:, P:256], ps[:, P:256],
                                                 tri_t)
                            nc.scalar.activation(et[:, P:256], ps[:, P:256],
                                                 AFT.Exp, scale=SCALE)
                        else:  # ti == 4*g + 3
                            nc.scalar.activation(et[:, P:256], ps[:, P:256],
                                                 AFT.Exp, scale=SCALE, bias=pb_t)
                        vt = vmem_t if is_mem else vnat[ti]
                        for sl in range(2):
                            if half_only and sl == 0:
                                continue
                            for hc in range(2):
                                nc.tensor.matmul(
                                    ph[(sl, hc)][:],
                                    et[:rows, sl * P:(sl + 1) * P],
                                    vt[:rows, hc * 512:(hc + 1) * 512],
                                    start=first, stop=last,
                                    skip_group_check=True)
                        nc.tensor.matmul(pd_row[0:1, c0:256],
                                         ones_col[:rows, :],
                                         et[:rows, c0:256], start=first,
                                         stop=last, skip_group_check=True)
                    # raw-copy h~ out of psum (releases banks immediately)
                    hs = {}
                    for sl in range(2):
                        for hc in range(2):
                            hsb = bst.tile([P, 512], F32, tag="hsb", bufs=8)
                            nc.vector.tensor_copy(hsb[:], ph[(sl, hc)][:])
                            hs[(sl, hc)] = hsb
                    # denominator [1,256] -> [128,2] via PE transpose, recip
                    pdsb = bst.tile([1, 256], F32, tag="pdsb")
                    nc.vector.tensor_copy(pdsb[:], pd_row[:])
                    pdt = bpd2.tile([P, 2], F32, tag="pdt")
                    for sl in range(2):
                        nc.tensor.transpose(pdt[:, sl:sl + 1],
                                            pdsb[0:1, sl * P:(sl + 1) * P],
                                            onesf[0:1, 0:1])
                    nc.vector.reciprocal(rden[:, g * 2:g * 2 + 2], pdt[:])
                    # combine: out = (h~raw*rden)*(alpha*w) - (alpha-1)*x
                    for sl in range(2):
                        p = 2 * g + sl
                        outb = bst.tile([P, H], F32, tag="outb")
                        for hc in range(2):
                            tmp = bst.tile([P, 512], F32, tag="tmp")
                            nc.vector.scalar_tensor_tensor(
                                tmp[:], hs[(sl, hc)][:],
                                rden[:, g * 2 + sl:g * 2 + sl + 1],
                                alpha[p][:, hc * 512:(hc + 1) * 512],
                                op0=ALU.mult, op1=ALU.mult)
                            sub_eng = nc.vector if g == 3 else nc.gpsimd
                            sub_eng.tensor_sub(
                                outb[:, hc * 512:(hc + 1) * 512], tmp[:],
                                xq[p][:, hc * 512:(hc + 1) * 512])
                        nc.sync.dma_start(out=out[p * P:(p + 1) * P, :],
                                          in_=outb[:])

    import os
    if os.environ.get("NO_WAIT_SPLIT") != "1":
        split_multi_waits(nc, limit=1, dma_limit=1)
    return nc


_NC_CACHE = None
_LAST_IN_MAPS = None


def _get_nc():
    global _NC_CACHE
    if _NC_CACHE is None:
        _NC_CACHE = build_nc()
    return _NC_CACHE


def prepare_in_maps(hidden_states, memory_state, q_w, k_w, norm_w, gate_w,
                    gate_b):
    hidden_states = np.asarray(hidden_states, dtype=np.float32)
    memory_state = np.asarray(memory_state, dtype=np.float32)
    q_w = np.asarray(q_w, dtype=np.float32)
    k_w = np.asarray(k_w, dtype=np.float32)
    norm_w = np.asarray(norm_w, dtype=np.float32)
    gate_w = np.asarray(gate_w, dtype=np.float32)
    gate_b = np.asarray(gate_b, dtype=np.float32)

    wsafe = np.where(norm_w == 0, np.float32(1.0), norm_w)
    a3 = np.ascontiguousarray(
        ((norm_w[:, None] * (q_w.T @ k_w)) * norm_w[None, :]).astype(NPBF))
    gw = np.ascontiguousarray(gate_w.T.astype(NPBF))
    tri = np.where(np.arange(P)[None, :] >= np.arange(P)[:, None],
                   np.float32(0.0), np.float32(NEG)).astype(np.float32)
    cbf = np.concatenate([np.ones((P, 1), NPBF),
                          np.broadcast_to(gate_b, (P, H)).astype(NPBF)], axis=1)
    cbf = np.ascontiguousarray(cbf)
    w_bc = np.ascontiguousarray(np.broadcast_to(norm_w, (P, H)).astype(NPBF))

    in_maps = []
    for c in range(N_CORES):
        b, par = divmod(c, 2)
        x16 = hidden_states[b].reshape(16, P, H)
        if par == 1:
            x16 = x16[[i ^ 1 for i in range(16)]]
        memb = memory_state[b] / wsafe[None, :]   # /w on both sides; *w folded
        cf32 = np.concatenate(
            [np.full((P, 1), NEG if par == 0 else 0.0, np.float32), tri], axis=1)
        in_maps.append({
            "x_all": np.ascontiguousarray(x16.reshape(S_FULL, H)
                                          .astype(np.float16)),
            "vmem": np.ascontiguousarray(memb.astype(NPBF)),
            "memT3": np.ascontiguousarray(memb.T.astype(NPBF)),
            "a3": a3, "gw": gw, "w_bc": w_bc,
            "cf32": np.ascontiguousarray(cf32), "cbf": cbf,
        })
    return in_maps


def kernel(**inputs):
    in_maps = prepare_in_maps(**inputs)
    global _LAST_IN_MAPS
    _LAST_IN_MAPS = in_maps
    nc = _get_nc()
    res = run_bass_kernel_spmd(nc, in_maps, list(range(N_CORES)))
    out = np.empty((B_FULL, S_FULL, H), dtype=np.float32)
    for c in range(N_CORES):
        b, par = divmod(c, 2)
        out[b].reshape(16, P, H)[par::2] = res.results[c]["out"].reshape(8, P, H)
    return out
